# revision 116
# baseline (speedup 1.0000x reference)
"""Supervised-contrastive point-cloud loss on 8 TRN2 NeuronCores.

Full inputs: features [8, 128, 4096] f32, labels_all [8, 4096] int.
Data-parallel: one cloud per core. Each core computes per-point losses
[128, 32]; the host averages (sum / N / B).

Math (per cloud, fmap [C=128, N=4096], labels [N], 16 classes):
  v = normalize(fmap.T)                 (rows unit-norm)
  E = exp(v @ v.T)                      (TEMP cancels in pos/(pos+neg))
  cst[i, c] = sum_{j: lab j == c} E[j, i]
  sel = cst[i, lab_i] ; tot = sum_c cst[i, c]
  A = sel - e ; B = tot - sel ; n = count[lab_i] ; nbar = N - n
  loss_i = ln(A*nbar + B*n) - ln(A*nbar)

E is symmetric, so only the upper block-triangle is computed (plus a
full-width bottom strip for passes >= B2, which trades ~2us of exp for
removing the tail's per-chunk transpose serialization). Pass b computes
gram rows for point-block b against columns [128b, 4096) in <=1024-wide
chunks: PE gram (bf16, f32 PSUM) -> exp -> SBUF bf16 -> DMA-engine xbar
transpose of the 128-col blocks (14 ns per 16x128 tile, on the otherwise
idle DMA path) -> PE class-sum matmuls. Class sums use the [128i, 16c]
output orientation (matmul cost scales with OUT free size, so 16-wide
outputs make the one-hot contraction ~free) and accumulate in two PSUM
banks (blocks 0-23 / 24-31; the first closes after pass 23 so 3/4 of the
sel/tot epilogue overlaps the tail):
  easy: cst[k] += e_blk(j in b, i in k) contraction over j (lhsT=e)
  hard: cst[b] += eT_blk(i in k, j in b) contraction over i (lhsT=eT)

The exp itself is split across engines: ACT runs plain Exp chunks
(0.833 ns/elem, no dtype speedup -- the kernel's wall); each pass's last
(diagonal-free) chunk is instead evaluated as exp(x) ~= P(x/2)^2 with a
fitted quadratic P: DVE does the PSUM->bf16 convert + 2 fused ALU
passes, Pool squares. Per-pass poly widths are sized so DVE+Pool time
stays under the pass's remaining ACT time.

Scheduling is counter-semaphore aware: the tile scheduler lowers cross-
engine deps into conservative engine-counter waits (an instruction waits
for ALL work emitted earlier on the dep engine), so emission order IS
the schedule: class-sum matmuls are deferred ~2 ACT-chunks (more for
poly chunks), one-hot/count setup drips a few instructions per
iteration, normalize runs a staged frontier 2 chunks ahead of its vn
multiplies, and deep e/eT pools keep WAR thresholds old.

PSUM: 3 rotating gram tiles [128,1024] f32 (2 banks each) + 2 cst banks
= 8 of 8 banks.
"""

import numpy as np
from contextlib import ExitStack

import concourse.bass as bass
import concourse.bacc as bacc
import concourse.bass_isa as bass_isa
import concourse.tile as tile
from concourse import mybir
from concourse.bass_utils import run_bass_kernel_spmd

F32 = mybir.dt.float32
BF16 = mybir.dt.bfloat16
I32 = mybir.dt.int32
AF = mybir.ActivationFunctionType
ALU = mybir.AluOpType
AX = mybir.AxisListType

B = 8
C = 128
N = 4096
NB = N // 128          # 32 point blocks of 128
NCLS = 16
CW = 1024              # exp / gram chunk width (2 PSUM banks; 3-deep
                       # gram rotation gives 2 chunks of WAR slack so the
                       # next gram hides even across ACT-idle poly chunks)
E_CONST = float(np.exp(1.0))


B2 = 26                # full-width cutover: passes >= B2 skip symmetry

# Quadratic fit P(x) ~= exp(x/2) on the off-diagonal gram range; then
# exp(x) ~= P(x)^2. Evaluated on DVE in 3 passes (stt, ts, tt) for chunks
# that contain no diagonal block, offloading ~1.4us of ACT per chunk:
#   u = (x + PS) * x ; p = PC * u + PA ; e = p * p
# Relative-weighted LS fit; |x| <= 0.75 covers unit-vector dots (~0.55 max).
_xg = np.linspace(-0.75, 0.75, 4001)
_pc, _pb, _pa = np.polyfit(_xg, np.exp(_xg / 2), 2, w=1.0 / np.exp(_xg / 2))
# Evaluated in y = x/2 (y = g scaled during the PSUM->SBUF convert pass;
# HW allows only ONE PSUM input per DVE op, so the stt must read the bf16
# SBUF copy): P = PA + 4*PC*(y^2 + PS2*y), exp(x) ~= P^2.
PS2 = float(_pb / (2 * _pc))
PC2 = float(4 * _pc)
PA = float(_pa)
def _poly_width(b):
    """Poly width per pass, sized so the pass's DVE (convert+2 ALU) and
    Pool (square) time stays below the pass's remaining ACT exp time.
    Passes 0-1 stay on ACT: the startup window already has DVE/Pool
    saturated by the normalize pipeline."""
    if b < 2:
        return 0
    if b <= 9:
        return 1024
    if b <= 15:
        return 640
    if b < B2:
        return 512
    return 0


def _square_split(b):
    """Fraction of the poly square done on DVE (rest on Pool). The square
    is all-SBUF so it is the one poly pass Pool may legally run (GPSIMD
    cannot access PSUM, which is why the convert stays on DVE)."""
    return 0.0


def _equal_split(v):
    m = -(-v // CW)
    nb = v // 128
    return sorted(128 * (nb // m + (1 if i < nb % m else 0)) for i in range(m))


def _chunk_list():
    """(pass b, start col, width, first col-block, n col-blocks, poly).

    Passes b < B2 cover cols [128b, N) (upper triangle; transposes feed the
    row-side class sums). Passes b >= B2 cover the full [128*B2, N) strip so
    the bottom-right corner needs no transposes or hard matmuls — removes
    the per-tiny-chunk HWDGE serialization at the tail for ~3us more exp.

    Widths are equal-split (ascending) so chunks stay >= ~1024 cols: an exp
    shorter than ~850ns can't hide the gram-of-next-chunk latency chain
    (prev exp end -> sem -> PE gram -> sem) and ACT bubbles.

    Passes 0..NPOLY_PASSES-1 end with a PW-wide poly chunk evaluated on DVE
    instead of ACT (diagonal-free by construction: pass-last has k0 > b).
    """
    chunks = []
    for b in range(NB):
        c0 = b * 128 if b < B2 else B2 * 128
        w_pass = N - c0
        pw = _poly_width(b) if b < B2 else 0
        sizes = [(w, False) for w in _equal_split(w_pass - pw)]
        if pw:
            sizes.append((pw, True))
        off = 0
        for w, pol in sizes:
            chunks.append((b, c0 + off, w, (c0 + off) // 128, w // 128, pol))
            off += w
    return chunks


def _body(ctx: ExitStack, tc: "tile.TileContext", feat, lab, outp):
    nc = tc.nc

    const = ctx.enter_context(tc.tile_pool(name="const", bufs=1))
    sb = ctx.enter_context(tc.tile_pool(name="sb", bufs=1))
    e_pool = ctx.enter_context(tc.tile_pool(name="e", bufs=24))
    et_pool = ctx.enter_context(tc.tile_pool(name="et", bufs=16))
    gp = ctx.enter_context(tc.tile_pool(name="gp", bufs=3, space="PSUM"))
    cstp = ctx.enter_context(tc.tile_pool(name="cstp", bufs=1, space="PSUM"))

    # Preload the one ACT table set that serves every function we use
    # (natural_log_exp_and_others: exp, ln, copy, identity) so the bacc
    # fixpoint pass doesn't insert per-function loads mid-kernel.
    from concourse.hw_specs import get_activation_tables

    tables = list(get_activation_tables(nc.m.arch).keys())
    nle_id = tables.index("natural_log_exp_and_others")
    tl = mybir.InstLoadActFuncSet(
        name=nc.get_next_instruction_name(), act_func_set_id=nle_id, ins=[], outs=[]
    )
    nc.scalar.add_instruction(tl)

    # ---------------- feature load: all chunks issued upfront --------------
    v_sb = sb.tile([128, N], F32, tag="v_sb")
    CS = 512
    dma_bounds = [0, 256, 512] + list(range(1024, N + 1, 512))
    for cl, ch in zip(dma_bounds[:-1], dma_bounds[1:]):
        nc.sync.dma_start(out=v_sb[:, cl:ch], in_=feat[:, cl:ch])

    # ---------------- normalize state (emitted lazily per 512-col chunk) ---
    # per chunk: vsq (DVE) -> ns partition-reduce (GPSIMD) -> ln (ACT)
    # -> rinv = exp(-0.5*ln) (ACT) -> vn = v * rinv_bc (DVE, bf16 out).
    # Lazy emission interleaves these with the first main-loop chunks so the
    # first exp starts as soon as vn[:, :1536] exists instead of after the
    # whole normalize.
    vsq = sb.tile([128, N], F32, tag="vsq")
    ns_all = sb.tile([128, N], F32, tag="ns_all")
    lns = vsq  # vsq is dead once its partition-reduce ran; ln reuses it
    rinv_bc = sb.tile([128, N], BF16, tag="rinv_bc")
    vn_bf = sb.tile([128, N], BF16, tag="vn_bf")
    norm_done = 0

    # The vn multiply of chunk k waits on rinv(k) from ACT; emitting it
    # right after vsq(k) would stall DVE's in-order queue and delay every
    # later vsq (whose Pool reduce and ACT ln sit behind it). The
    # vsq->reduce->ln->rinv frontier therefore runs ~2 chunks ahead of the
    # vn emission that consumers actually need.
    stage_done = 0
    _vn_pending = []

    def _stages_to(col):
        nonlocal stage_done
        while stage_done < min(col, N):
            # 512-col chunks for the startup-critical first 2048 cols,
            # 1024 after (halves the ACT instruction-init overhead)
            cl = stage_done
            ch = cl + (256 if cl < 512 else (CS if cl < 2048 else 1024))
            nc.vector.tensor_mul(vsq[:, cl:ch], v_sb[:, cl:ch], v_sb[:, cl:ch])
            nc.gpsimd.partition_all_reduce(
                ns_all[:, cl:ch], vsq[:, cl:ch], channels=128,
                reduce_op=bass_isa.ReduceOp.add,
            )
            nc.scalar.activation(lns[:, cl:ch], ns_all[:, cl:ch], AF.Ln)
            nc.scalar.activation(rinv_bc[:, cl:ch], lns[:, cl:ch], AF.Exp, scale=-0.5)
            _vn_pending.append((cl, ch))
            stage_done = ch

    def ensure_vn(col):
        nonlocal norm_done
        col = min(col, N)
        _stages_to(min(col + 2048, N))
        while norm_done < col:
            cl, ch = _vn_pending.pop(0)
            nc.vector.tensor_mul(vn_bf[:, cl:ch], v_sb[:, cl:ch], rinv_bc[:, cl:ch])
            norm_done = ch

    # ---------------- constants ----------------
    iota_i = const.tile([128, NCLS], I32, tag="iota_i")
    nc.gpsimd.iota(iota_i, pattern=[[1, NCLS]], base=0, channel_multiplier=0)
    iota_f = const.tile([128, NCLS], F32, tag="iota_f")
    nc.vector.tensor_copy(iota_f, iota_i)

    # ---------------- labels -> one-hot (emitted in two batches) -----------
    labels_sb = sb.tile([128, NB], F32, tag="labels_sb")
    nc.gpsimd.dma_start(out=labels_sb, in_=lab[:, :])

    oh_f = sb.tile([128, NB * NCLS], F32, tag="oh_f")  # [128, 512]
    oh_b = sb.tile([128, NB * NCLS], BF16, tag="oh_b")
    oh_done = 0

    def ensure_oh(kmax):
        nonlocal oh_done
        if oh_done > kmax:
            return
        lo = oh_done
        for b in range(lo, kmax + 1):
            nc.vector.tensor_scalar(
                out=oh_f[:, b * NCLS : (b + 1) * NCLS],
                in0=iota_f,
                scalar1=labels_sb[:, b : b + 1],
                scalar2=None,
                op0=ALU.is_equal,
            )
        nc.vector.tensor_copy(
            oh_b[:, lo * NCLS : (kmax + 1) * NCLS],
            oh_f[:, lo * NCLS : (kmax + 1) * NCLS],
        )
        oh_done = kmax + 1

    # Class counts / per-point counts. The tile scheduler lowers cross-
    # engine deps into conservative engine-counter semaphores (a PE
    # instruction waits for ALL DVE work emitted before it), so this ~37-
    # instruction batch is dripped a few instructions per iteration via a
    # chore queue instead of being emitted in one lump that would gate
    # every later gram on its completion.
    n_row = sb.tile([128, NB], F32, tag="n_row")
    nbar = sb.tile([128, NB], F32, tag="nbar")
    cnt_all = sb.tile([128, NB * NCLS], F32, tag="cnt_all")
    n_bc = sb.tile([128, NCLS], F32, tag="n_bc")
    n_rep = sb.tile([128, NB * NCLS], F32, tag="n_rep")
    nrm = sb.tile([128, NB * NCLS], F32, tag="nrm")

    def _counts_chores():
        for k in range(11, NB, 4):
            yield lambda k=k: ensure_oh(min(k + 3, NB - 1))
        yield lambda: nc.gpsimd.partition_all_reduce(
            cnt_all, oh_f, channels=128, reduce_op=bass_isa.ReduceOp.add
        )
        yield lambda: nc.vector.tensor_reduce(
            out=n_bc,
            in_=cnt_all.rearrange("p (b c) -> p c b", c=NCLS),
            axis=AX.X,
            op=ALU.add,
        )
        for b in range(NB):
            yield lambda b=b: nc.gpsimd.tensor_copy(
                n_rep[:, b * NCLS : (b + 1) * NCLS], n_bc
            )
        yield lambda: nc.gpsimd.tensor_mul(nrm, oh_f, n_rep)
        yield lambda: nc.vector.tensor_reduce(
            out=n_row,
            in_=nrm.rearrange("p (b c) -> p b c", c=NCLS),
            axis=AX.X,
            op=ALU.add,
        )
        yield lambda: nc.gpsimd.tensor_scalar(
            out=nbar, in0=n_row, scalar1=-1.0, scalar2=float(N),
            op0=ALU.mult, op1=ALU.add,
        )

    # ---------------- main loop: gram -> exp -> transpose -> class sums ----
    chunks = _chunk_list()
    nch = len(chunks)
    pp = ctx.enter_context(tc.tile_pool(name="pp", bufs=4))
    is_poly = [c[5] for c in chunks]
    # Class sums accumulate in TWO PSUM banks: cst_t[0] holds blocks 0-23
    # (cols k*16..), cst_t[1] blocks 24-31. Each bank is one zero region /
    # one accumulation group: its first matmul starts the group (lazily
    # zeroing the bank), every later matmul accumulates (first touch of
    # still-pending bytes initializes), and its last matmul closes it.
    # Bank 0 closes after pass 23's hards, so 3/4 of the sel/tot epilogue
    # overlaps the tail passes (PSUM reads are rejected mid-group).
    cst_t = [
        cstp.tile([128, 512], F32, tag="cst0", name="cst0"),
        cstp.tile([128, 512], F32, tag="cst1", name="cst1"),
    ]
    CSPLIT = 24

    def _bank(k):
        return (0, k) if k < CSPLIT else (1, k - CSPLIT)

    def cst_slice(k):
        bk, kk = _bank(k)
        return cst_t[bk][:, kk * NCLS : (kk + 1) * NCLS]

    # dry pass over the emission schedule to find each bank's first/last
    # matmul (emission order == PE execution order)
    bank_n = [0, 0]
    for b, _, _, k0, nk, _pol in chunks:
        for t in range(nk):
            bank_n[_bank(k0 + t)[0]] += 1
        ntb = (nk - 1 if k0 == b else nk) if b < B2 else 0
        bank_n[_bank(b)[0]] += ntb
    bank_idx = [0, 0]

    def cst_flags(k_or_b):
        bk = _bank(k_or_b)[0]
        st = bank_idx[bk] == 0
        sp = bank_idx[bk] == bank_n[bk] - 1
        bank_idx[bk] += 1
        return st, sp

    e_tiles = [None] * nch
    et_tiles = [None] * nch

    def oh_slice(k):
        return oh_b[:, k * NCLS : (k + 1) * NCLS]

    g_tiles = [None] * nch
    p_tiles = [None] * nch

    def emit_transpose(ci):
        b, c0, w, k0, nk, _pol = chunks[ci]
        # transpose the strictly-upper col-blocks (skip the diagonal block);
        # full-width passes (b >= B2) need no transposes at all
        ntb = (nk - 1 if k0 == b else nk) if b < B2 else 0
        if ntb > 0:
            skip = 128 if k0 == b else 0
            et = et_pool.tile([128, CW], BF16, tag="et", name=f"et{ci}")
            nc.sync.dma_start_transpose(
                et[:, : ntb * 128].rearrange("p (t f) -> p t f", f=128),
                e_tiles[ci][:, skip : skip + ntb * 128],
            )
            et_tiles[ci] = et

    def emit_gram_exp(ci):
        b, c0, w, k0, nk, _pol = chunks[ci]
        ensure_vn(c0 + w)
        g = gp.tile([128, CW], F32, tag="g", name=f"g{ci}")
        g_tiles[ci] = g
        for q in range(0, w, 512):
            qw = min(512, w - q)
            nc.tensor.matmul(
                g[:, q : q + qw],
                lhsT=vn_bf[:, b * 128 : (b + 1) * 128],
                rhs=vn_bf[:, c0 + q : c0 + q + qw],
                start=True,
                stop=True,
            )
        if ci + 1 < nch:
            # prefetch the next chunk's vn AFTER this gram (so the gram's
            # counter-sem threshold excludes it) but BEFORE this exp (so
            # the ln/rinv execute under it instead of serializing after)
            nb_, nc0, nw = chunks[ci + 1][0:3]
            ensure_vn(nc0 + nw)
        if not is_poly[ci]:
            e = e_pool.tile([128, CW], BF16, tag="e", name=f"e{ci}")
            nc.scalar.activation(e[:, :w], g[:, :w], AF.Exp)
            e_tiles[ci] = e
            emit_transpose(ci)

    def emit_poly(ci):
        # deferred ~2 iterations past the gram so the DVE work doesn't
        # inflate the counter-semaphore thresholds of the next grams;
        # the final square runs on the idle Pool engine (or split DVE/Pool)
        b, c0, w, k0, nk, _pol = chunks[ci]
        g = g_tiles[ci]
        y = pp.tile([128, CW], BF16, tag="py", name=f"py{ci}")
        nc.vector.tensor_scalar(
            out=y[:, :w], in0=g[:, :w], scalar1=0.5, scalar2=None,
            op0=ALU.mult,
        )
        u = pp.tile([128, CW], BF16, tag="pu", name=f"pu{ci}")
        nc.vector.scalar_tensor_tensor(
            out=u[:, :w], in0=y[:, :w], scalar=PS2, in1=y[:, :w],
            op0=ALU.add, op1=ALU.mult,
        )
        p = pp.tile([128, CW], BF16, tag="ppt", name=f"ppt{ci}")
        nc.vector.tensor_scalar(
            out=p[:, :w], in0=u[:, :w], scalar1=PC2, scalar2=PA,
            op0=ALU.mult, op1=ALU.add,
        )
        p_tiles[ci] = p

    def emit_poly_sq(ci):
        # square + transpose one slot later still: keeps the Pool-counter
        # thresholds of later-emitted exps a pass older in the thin-window
        # tail region
        b, c0, w, k0, nk, _pol = chunks[ci]
        p = p_tiles[ci]
        e = e_pool.tile([128, CW], BF16, tag="e", name=f"e{ci}")
        wd = int(w * _square_split(b)) // 128 * 128
        if wd > 0:
            nc.vector.tensor_mul(e[:, :wd], p[:, :wd], p[:, :wd])
        if wd < w:
            nc.gpsimd.tensor_mul(e[:, wd:w], p[:, wd:w], p[:, wd:w])
        e_tiles[ci] = e
        emit_transpose(ci)

    def emit_easy(ci):
        b, c0, w, k0, nk, _pol = chunks[ci]
        ensure_oh(b)
        for t in range(nk):
            k = k0 + t
            st, sp = cst_flags(k)
            nc.tensor.matmul(
                cst_slice(k),
                lhsT=e_tiles[ci][:, t * 128 : (t + 1) * 128],
                rhs=oh_slice(b),
                start=st,
                stop=sp,
            )

    def emit_hard(ci):
        b, c0, w, k0, nk, _pol = chunks[ci]
        ntb = nk - 1 if k0 == b else nk
        kfirst = b + 1 if k0 == b else k0
        ensure_oh(kfirst + ntb - 1)
        for t in range(ntb):
            k = kfirst + t
            st, sp = cst_flags(b)
            nc.tensor.matmul(
                cst_slice(b),
                lhsT=et_tiles[ci][:, t * 128 : (t + 1) * 128],
                rhs=oh_slice(k),
                start=st,
                stop=sp,
            )

    # ---------------- phased epilogue: sel/tot per 16-block half -----------
    masked = sb.tile([128, NB * NCLS], F32, tag="masked")
    sel = sb.tile([128, NB], F32, tag="sel")
    tot = sb.tile([128, NB], F32, tag="tot")

    a_t = sb.tile([128, NB], F32, tag="a_t")
    b_t = sb.tile([128, NB], F32, tag="b_t")
    num = sb.tile([128, NB], F32, tag="num")
    den = sb.tile([128, NB], F32, tag="den")
    l_den = sb.tile([128, NB], F32, tag="l_den")
    l_num = sb.tile([128, NB], F32, tag="l_num")
    lt = sb.tile([128, NB], F32, tag="lt")

    def _seltot_chores(h):
        nblk = CSPLIT if h == 0 else NB - CSPLIT
        lo, hi = (0, CSPLIT * NCLS) if h == 0 else (CSPLIT * NCLS, NB * NCLS)
        bs = slice(0, CSPLIT) if h == 0 else slice(CSPLIT, NB)
        cs = cst_t[h][:, 0 : nblk * NCLS]
        yield lambda: nc.vector.tensor_mul(masked[:, lo:hi], cs, oh_f[:, lo:hi])
        yield lambda: nc.vector.tensor_reduce(
            out=sel[:, bs],
            in_=masked[:, lo:hi].rearrange("p (b c) -> p b c", c=NCLS),
            axis=AX.X,
            op=ALU.add,
        )
        yield lambda: nc.vector.tensor_reduce(
            out=tot[:, bs],
            in_=cs.rearrange("p (b c) -> p b c", c=NCLS),
            axis=AX.X,
            op=ALU.add,
        )
        yield lambda: nc.vector.tensor_scalar_add(a_t[:, bs], sel[:, bs], -E_CONST)
        yield lambda: nc.vector.tensor_sub(b_t[:, bs], tot[:, bs], sel[:, bs])
        yield lambda: nc.vector.tensor_mul(num[:, bs], a_t[:, bs], nbar[:, bs])
        def fin1():
            nc.vector.tensor_mul(den[:, bs], b_t[:, bs], n_row[:, bs])
            nc.vector.tensor_add(den[:, bs], den[:, bs], num[:, bs])
        yield fin1
        def fin2():
            nc.scalar.activation(l_den[:, bs], den[:, bs], AF.Ln)
            nc.scalar.activation(l_num[:, bs], num[:, bs], AF.Ln)
            nc.vector.tensor_sub(lt[:, bs], l_den[:, bs], l_num[:, bs])
            nc.sync.dma_start(out=outp[:, bs], in_=lt[:, bs])
        yield fin2

    def emit_seltot(h):
        for chore in _seltot_chores(h):
            chore()

    ensure_oh(0)  # block 0's one-hot before any vn work lands on DVE

    # Deferred cs-matmul schedule: poly chunks produce e ~3us later than
    # ACT chunks, so their easy/hard emission is pushed further out to keep
    # the in-order PE queue from blocking the next gram (which stalls ACT).
    # Deferral is measured in ACT chunks, not raw chunk index: easy(cj) must
    # sit behind >= 2 further exps (and poly output behind the DVE latency)
    # when PE reaches it, else its wait blocks the next gram and stalls ACT.
    NSLOT = nch + 8
    easy_due = [[] for _ in range(NSLOT)]
    hard_due = [[] for _ in range(NSLOT)]
    act_idx = [ci for ci in range(nch) if not is_poly[ci]]
    hard_slot_of = {}
    for cj in range(nch):
        later = [ci for ci in act_idx if ci > cj]
        ne, nh = (2, 5) if is_poly[cj] else (1, 4)
        easy_slot = later[ne] if len(later) > ne else nch + 1
        hard_slot = later[nh] if len(later) > nh else nch + 2
        easy_due[easy_slot].append(cj)
        hard_due[hard_slot].append(cj)
        hard_slot_of[cj] = hard_slot

    # bank 0 (blocks 0-15) closes once the hard matmuls of the last chunk
    # of pass 15 are emitted
    last_chunk_of_pass = {}
    for ci, (b, _, _, _, _, _pol) in enumerate(chunks):
        last_chunk_of_pass[b] = ci
    half0_at = hard_slot_of[last_chunk_of_pass[CSPLIT - 1]]

    poly_due = [[] for _ in range(NSLOT)]
    polysq_due = [[] for _ in range(NSLOT)]
    for cj in range(nch):
        if is_poly[cj]:
            poly_due[cj + 2].append(cj)
            polysq_due[cj + 3].append(cj)

    chores = list(_counts_chores())
    chore_pos = 0

    for ci in range(NSLOT):
        if ci < nch:
            emit_gram_exp(ci)
        for cj in poly_due[ci]:
            emit_poly(cj)
        for cj in polysq_due[ci]:
            emit_poly_sq(cj)
        if ci >= 6:
            for _ in range(2):
                if chore_pos < len(chores):
                    chores[chore_pos]()
                    chore_pos += 1
        for cj in easy_due[ci]:
            emit_easy(cj)
        for cj in hard_due[ci]:
            if et_tiles[cj] is not None:
                emit_hard(cj)
        if ci == half0_at:
            chores.extend(_seltot_chores(0))
    while chore_pos < len(chores):
        chores[chore_pos]()
        chore_pos += 1
    emit_seltot(1)


def build_nc():
    nc = bacc.Bacc()
    feat = nc.declare_dram_parameter("features", [C, N], F32, isOutput=False)
    lab = nc.declare_dram_parameter("labels", [128, NB], F32, isOutput=False)
    outp = nc.declare_dram_parameter("out", [128, NB], F32, isOutput=True)
    with tile.TileContext(nc) as tc:
        with ExitStack() as ctx:
            _body(ctx, tc, feat[:, :], lab[:, :], outp)
    nc.finalize()
    return nc


_NC_CACHE = None


def _get_nc():
    global _NC_CACHE
    if _NC_CACHE is None:
        _NC_CACHE = build_nc()
    return _NC_CACHE


def make_in_maps(features: np.ndarray, labels_all: np.ndarray):
    in_maps = []
    for i in range(B):
        f = np.ascontiguousarray(features[i], dtype=np.float32)
        # labels_sb[p, b] = labels[128*b + p]
        l = np.ascontiguousarray(
            labels_all[i].astype(np.float32).reshape(NB, 128).T
        )
        in_maps.append({"features": f, "labels": l})
    return in_maps


def kernel(features: np.ndarray, labels_all: np.ndarray) -> np.ndarray:
    nc = _get_nc()
    in_maps = make_in_maps(features, labels_all)
    r = run_bass_kernel_spmd(nc, in_maps, core_ids=list(range(B)))
    sums = np.array(
        [np.sum(r.results[i]["out"], dtype=np.float64) for i in range(B)]
    )
    return np.float32(np.mean(sums) / N)


# revision 119
# speedup vs baseline: 1.0015x; 1.0015x over previous
"""Supervised-contrastive point-cloud loss on 8 TRN2 NeuronCores.

Full inputs: features [8, 128, 4096] f32, labels_all [8, 4096] int.
Data-parallel: one cloud per core. Each core computes per-point losses
[128, 32]; the host averages (sum / N / B).

Math (per cloud, fmap [C=128, N=4096], labels [N], 16 classes):
  v = normalize(fmap.T)                 (rows unit-norm)
  E = exp(v @ v.T)                      (TEMP cancels in pos/(pos+neg))
  cst[i, c] = sum_{j: lab j == c} E[j, i]
  sel = cst[i, lab_i] ; tot = sum_c cst[i, c]
  A = sel - e ; B = tot - sel ; n = count[lab_i] ; nbar = N - n
  loss_i = ln(A*nbar + B*n) - ln(A*nbar)

E is symmetric, so only the upper block-triangle is computed (plus a
full-width bottom strip for passes >= B2, which trades ~2us of exp for
removing the tail's per-chunk transpose serialization). Pass b computes
gram rows for point-block b against columns [128b, 4096) in <=1024-wide
chunks: PE gram (bf16, f32 PSUM) -> exp -> SBUF bf16 -> DMA-engine xbar
transpose of the 128-col blocks (14 ns per 16x128 tile, on the otherwise
idle DMA path) -> PE class-sum matmuls. Class sums use the [128i, 16c]
output orientation (matmul cost scales with OUT free size, so 16-wide
outputs make the one-hot contraction ~free) and accumulate in two PSUM
banks (blocks 0-23 / 24-31; the first closes after pass 23 so 3/4 of the
sel/tot epilogue overlaps the tail):
  easy: cst[k] += e_blk(j in b, i in k) contraction over j (lhsT=e)
  hard: cst[b] += eT_blk(i in k, j in b) contraction over i (lhsT=eT)

The exp itself is split across engines: ACT runs plain Exp chunks
(0.833 ns/elem, no dtype speedup -- the kernel's wall); each pass's last
(diagonal-free) chunk is instead evaluated as exp(x) ~= P(x/2)^2 with a
fitted quadratic P: DVE does the PSUM->bf16 convert + 2 fused ALU
passes, Pool squares. Per-pass poly widths are sized so DVE+Pool time
stays under the pass's remaining ACT time.

Scheduling is counter-semaphore aware: the tile scheduler lowers cross-
engine deps into conservative engine-counter waits (an instruction waits
for ALL work emitted earlier on the dep engine), so emission order IS
the schedule: class-sum matmuls are deferred ~2 ACT-chunks (more for
poly chunks), one-hot/count setup drips a few instructions per
iteration, normalize runs a staged frontier 2 chunks ahead of its vn
multiplies, and deep e/eT pools keep WAR thresholds old.

PSUM: 3 rotating gram tiles [128,1024] f32 (2 banks each) + 2 cst banks
= 8 of 8 banks.
"""

import numpy as np
from contextlib import ExitStack

import concourse.bass as bass
import concourse.bacc as bacc
import concourse.bass_isa as bass_isa
import concourse.tile as tile
from concourse import mybir
from concourse.bass_utils import run_bass_kernel_spmd

F32 = mybir.dt.float32
BF16 = mybir.dt.bfloat16
I32 = mybir.dt.int32
AF = mybir.ActivationFunctionType
ALU = mybir.AluOpType
AX = mybir.AxisListType

B = 8
C = 128
N = 4096
NB = N // 128          # 32 point blocks of 128
NCLS = 16
CW = 1024              # exp / gram chunk width (2 PSUM banks; 3-deep
                       # gram rotation gives 2 chunks of WAR slack so the
                       # next gram hides even across ACT-idle poly chunks)
E_CONST = float(np.exp(1.0))


B2 = 26                # full-width cutover: passes >= B2 skip symmetry

# Quadratic fit P(x) ~= exp(x/2) on the off-diagonal gram range; then
# exp(x) ~= P(x)^2. Evaluated on DVE in 3 passes (stt, ts, tt) for chunks
# that contain no diagonal block, offloading ~1.4us of ACT per chunk:
#   u = (x + PS) * x ; p = PC * u + PA ; e = p * p
# Relative-weighted LS fit; |x| <= 0.75 covers unit-vector dots (~0.55 max).
_xg = np.linspace(-0.75, 0.75, 4001)
_pc, _pb, _pa = np.polyfit(_xg, np.exp(_xg / 2), 2, w=1.0 / np.exp(_xg / 2))
# Evaluated in y = x/2 (y = g scaled during the PSUM->SBUF convert pass;
# HW allows only ONE PSUM input per DVE op, so the stt must read the bf16
# SBUF copy): P = PA + 4*PC*(y^2 + PS2*y), exp(x) ~= P^2.
PS2 = float(_pb / (2 * _pc))
PC2 = float(4 * _pc)
PA = float(_pa)
def _poly_width(b):
    """Poly width per pass, sized so the pass's DVE (convert+2 ALU) and
    Pool (square) time stays below the pass's remaining ACT exp time.
    Passes 0-1 stay on ACT: the startup window already has DVE/Pool
    saturated by the normalize pipeline."""
    if b < 2:
        return 0
    if b <= 9:
        return 1024
    if b <= 15:
        return 640
    if b < B2:
        return 512
    return 0


def _square_split(b):
    """Fraction of the poly square done on DVE (rest on Pool). The square
    is all-SBUF so it is the one poly pass Pool may legally run (GPSIMD
    cannot access PSUM, which is why the convert stays on DVE)."""
    return 0.0


def _equal_split(v):
    m = -(-v // CW)
    nb = v // 128
    return sorted(128 * (nb // m + (1 if i < nb % m else 0)) for i in range(m))


def _chunk_list():
    """(pass b, start col, width, first col-block, n col-blocks, poly).

    Passes b < B2 cover cols [128b, N) (upper triangle; transposes feed the
    row-side class sums). Passes b >= B2 cover the full [128*B2, N) strip so
    the bottom-right corner needs no transposes or hard matmuls — removes
    the per-tiny-chunk HWDGE serialization at the tail for ~3us more exp.

    Widths are equal-split (ascending) so chunks stay >= ~1024 cols: an exp
    shorter than ~850ns can't hide the gram-of-next-chunk latency chain
    (prev exp end -> sem -> PE gram -> sem) and ACT bubbles.

    Passes 0..NPOLY_PASSES-1 end with a PW-wide poly chunk evaluated on DVE
    instead of ACT (diagonal-free by construction: pass-last has k0 > b).
    """
    chunks = []
    for b in range(NB):
        c0 = b * 128 if b < B2 else B2 * 128
        w_pass = N - c0
        pw = _poly_width(b) if b < B2 else 0
        sizes = [(w, False) for w in _equal_split(w_pass - pw)]
        if pw:
            sizes.append((pw, True))
        off = 0
        for w, pol in sizes:
            chunks.append((b, c0 + off, w, (c0 + off) // 128, w // 128, pol))
            off += w
    return chunks


def _body(ctx: ExitStack, tc: "tile.TileContext", feat, lab, outp):
    nc = tc.nc

    const = ctx.enter_context(tc.tile_pool(name="const", bufs=1))
    sb = ctx.enter_context(tc.tile_pool(name="sb", bufs=1))
    e_pool = ctx.enter_context(tc.tile_pool(name="e", bufs=24))
    et_pool = ctx.enter_context(tc.tile_pool(name="et", bufs=16))
    gp = ctx.enter_context(tc.tile_pool(name="gp", bufs=3, space="PSUM"))
    cstp = ctx.enter_context(tc.tile_pool(name="cstp", bufs=1, space="PSUM"))

    # Preload the one ACT table set that serves every function we use
    # (natural_log_exp_and_others: exp, ln, copy, identity) so the bacc
    # fixpoint pass doesn't insert per-function loads mid-kernel.
    from concourse.hw_specs import get_activation_tables

    tables = list(get_activation_tables(nc.m.arch).keys())
    nle_id = tables.index("natural_log_exp_and_others")
    tl = mybir.InstLoadActFuncSet(
        name=nc.get_next_instruction_name(), act_func_set_id=nle_id, ins=[], outs=[]
    )
    nc.scalar.add_instruction(tl)

    # ---------------- feature load: all chunks issued upfront --------------
    v_sb = sb.tile([128, N], F32, tag="v_sb")
    CS = 512
    dma_bounds = [0, 256, 512] + list(range(1024, N + 1, 512))
    for cl, ch in zip(dma_bounds[:-1], dma_bounds[1:]):
        nc.sync.dma_start(out=v_sb[:, cl:ch], in_=feat[:, cl:ch])

    # ---------------- normalize state (emitted lazily per 512-col chunk) ---
    # per chunk: vsq (DVE) -> ns partition-reduce (GPSIMD) -> ln (ACT)
    # -> rinv = exp(-0.5*ln) (ACT) -> vn = v * rinv_bc (DVE, bf16 out).
    # Lazy emission interleaves these with the first main-loop chunks so the
    # first exp starts as soon as vn[:, :1536] exists instead of after the
    # whole normalize.
    vsq = sb.tile([128, N], F32, tag="vsq")
    ns_all = sb.tile([128, N], F32, tag="ns_all")
    lns = vsq  # vsq is dead once its partition-reduce ran; ln reuses it
    rinv_bc = sb.tile([128, N], BF16, tag="rinv_bc")
    vn_bf = sb.tile([128, N], BF16, tag="vn_bf")
    norm_done = 0

    # The vn multiply of chunk k waits on rinv(k) from ACT; emitting it
    # right after vsq(k) would stall DVE's in-order queue and delay every
    # later vsq (whose Pool reduce and ACT ln sit behind it). The
    # vsq->reduce->ln->rinv frontier therefore runs ~2 chunks ahead of the
    # vn emission that consumers actually need.
    stage_done = 0
    _vn_pending = []

    def _stages_to(col):
        nonlocal stage_done
        while stage_done < min(col, N):
            # 512-col chunks for the startup-critical first 2048 cols,
            # 1024 after (halves the ACT instruction-init overhead)
            cl = stage_done
            ch = cl + (256 if cl < 512 else (CS if cl < 2048 else 1024))
            nc.vector.tensor_mul(vsq[:, cl:ch], v_sb[:, cl:ch], v_sb[:, cl:ch])
            nc.gpsimd.partition_all_reduce(
                ns_all[:, cl:ch], vsq[:, cl:ch], channels=128,
                reduce_op=bass_isa.ReduceOp.add,
            )
            nc.scalar.activation(lns[:, cl:ch], ns_all[:, cl:ch], AF.Ln)
            nc.scalar.activation(rinv_bc[:, cl:ch], lns[:, cl:ch], AF.Exp, scale=-0.5)
            _vn_pending.append((cl, ch))
            stage_done = ch

    def ensure_vn(col):
        nonlocal norm_done
        col = min(col, N)
        _stages_to(min(col + 2048, N))
        while norm_done < col:
            cl, ch = _vn_pending.pop(0)
            nc.vector.tensor_mul(vn_bf[:, cl:ch], v_sb[:, cl:ch], rinv_bc[:, cl:ch])
            norm_done = ch

    # ---------------- constants ----------------
    iota_i = const.tile([128, NCLS], I32, tag="iota_i")
    nc.gpsimd.iota(iota_i, pattern=[[1, NCLS]], base=0, channel_multiplier=0)
    iota_f = const.tile([128, NCLS], F32, tag="iota_f")
    nc.vector.tensor_copy(iota_f, iota_i)

    # ---------------- labels -> one-hot (emitted in two batches) -----------
    labels_sb = sb.tile([128, NB], F32, tag="labels_sb")
    nc.gpsimd.dma_start(out=labels_sb, in_=lab[:, :])

    oh_f = sb.tile([128, NB * NCLS], F32, tag="oh_f")  # [128, 512]
    oh_b = sb.tile([128, NB * NCLS], BF16, tag="oh_b")
    oh_done = 0

    def ensure_oh(kmax):
        nonlocal oh_done
        if oh_done > kmax:
            return
        lo = oh_done
        for b in range(lo, kmax + 1):
            nc.vector.tensor_scalar(
                out=oh_f[:, b * NCLS : (b + 1) * NCLS],
                in0=iota_f,
                scalar1=labels_sb[:, b : b + 1],
                scalar2=None,
                op0=ALU.is_equal,
            )
        nc.vector.tensor_copy(
            oh_b[:, lo * NCLS : (kmax + 1) * NCLS],
            oh_f[:, lo * NCLS : (kmax + 1) * NCLS],
        )
        oh_done = kmax + 1

    # Class counts / per-point counts. The tile scheduler lowers cross-
    # engine deps into conservative engine-counter semaphores (a PE
    # instruction waits for ALL DVE work emitted before it), so this ~37-
    # instruction batch is dripped a few instructions per iteration via a
    # chore queue instead of being emitted in one lump that would gate
    # every later gram on its completion.
    n_row = sb.tile([128, NB], F32, tag="n_row")
    nbar = sb.tile([128, NB], F32, tag="nbar")
    cnt_all = sb.tile([128, NB * NCLS], F32, tag="cnt_all")
    n_bc = sb.tile([128, NCLS], F32, tag="n_bc")
    n_rep = sb.tile([128, NB * NCLS], F32, tag="n_rep")
    nrm = sb.tile([128, NB * NCLS], F32, tag="nrm")

    def _counts_chores():
        for k in range(11, NB, 4):
            yield lambda k=k: ensure_oh(min(k + 3, NB - 1))
        yield lambda: nc.gpsimd.partition_all_reduce(
            cnt_all, oh_f, channels=128, reduce_op=bass_isa.ReduceOp.add
        )
        yield lambda: nc.vector.tensor_reduce(
            out=n_bc,
            in_=cnt_all.rearrange("p (b c) -> p c b", c=NCLS),
            axis=AX.X,
            op=ALU.add,
        )
        for b in range(NB):
            yield lambda b=b: nc.gpsimd.tensor_copy(
                n_rep[:, b * NCLS : (b + 1) * NCLS], n_bc
            )
        yield lambda: nc.gpsimd.tensor_mul(nrm, oh_f, n_rep)
        yield lambda: nc.vector.tensor_reduce(
            out=n_row,
            in_=nrm.rearrange("p (b c) -> p b c", c=NCLS),
            axis=AX.X,
            op=ALU.add,
        )
        yield lambda: nc.gpsimd.tensor_scalar(
            out=nbar, in0=n_row, scalar1=-1.0, scalar2=float(N),
            op0=ALU.mult, op1=ALU.add,
        )

    # ---------------- main loop: gram -> exp -> transpose -> class sums ----
    chunks = _chunk_list()
    nch = len(chunks)
    pp = ctx.enter_context(tc.tile_pool(name="pp", bufs=4))
    is_poly = [c[5] for c in chunks]
    # Class sums accumulate in TWO PSUM banks: cst_t[0] holds blocks 0-23
    # (cols k*16..), cst_t[1] blocks 24-31. Each bank is one zero region /
    # one accumulation group: its first matmul starts the group (lazily
    # zeroing the bank), every later matmul accumulates (first touch of
    # still-pending bytes initializes), and its last matmul closes it.
    # Bank 0 closes after pass 23's hards, so 3/4 of the sel/tot epilogue
    # overlaps the tail passes (PSUM reads are rejected mid-group).
    cst_t = [
        cstp.tile([128, 512], F32, tag="cst0", name="cst0"),
        cstp.tile([128, 512], F32, tag="cst1", name="cst1"),
    ]
    CSPLIT = 24

    def _bank(k):
        return (0, k) if k < CSPLIT else (1, k - CSPLIT)

    def cst_slice(k):
        bk, kk = _bank(k)
        return cst_t[bk][:, kk * NCLS : (kk + 1) * NCLS]

    # dry pass over the emission schedule to find each bank's first/last
    # matmul (emission order == PE execution order)
    bank_n = [0, 0]
    for b, _, _, k0, nk, _pol in chunks:
        for t in range(nk):
            bank_n[_bank(k0 + t)[0]] += 1
        ntb = (nk - 1 if k0 == b else nk) if b < B2 else 0
        bank_n[_bank(b)[0]] += ntb
    bank_idx = [0, 0]

    def cst_flags(k_or_b):
        bk = _bank(k_or_b)[0]
        st = bank_idx[bk] == 0
        sp = bank_idx[bk] == bank_n[bk] - 1
        bank_idx[bk] += 1
        return st, sp

    e_tiles = [None] * nch
    et_tiles = [None] * nch

    def oh_slice(k):
        return oh_b[:, k * NCLS : (k + 1) * NCLS]

    g_tiles = [None] * nch
    p_tiles = [None] * nch

    def emit_transpose(ci):
        b, c0, w, k0, nk, _pol = chunks[ci]
        # transpose the strictly-upper col-blocks (skip the diagonal block);
        # full-width passes (b >= B2) need no transposes at all
        ntb = (nk - 1 if k0 == b else nk) if b < B2 else 0
        if ntb > 0:
            skip = 128 if k0 == b else 0
            et = et_pool.tile([128, CW], BF16, tag="et", name=f"et{ci}")
            nc.sync.dma_start_transpose(
                et[:, : ntb * 128].rearrange("p (t f) -> p t f", f=128),
                e_tiles[ci][:, skip : skip + ntb * 128],
            )
            et_tiles[ci] = et

    def emit_gram_exp(ci):
        b, c0, w, k0, nk, _pol = chunks[ci]
        ensure_vn(c0 + w)
        g = gp.tile([128, CW], F32, tag="g", name=f"g{ci}")
        g_tiles[ci] = g
        for q in range(0, w, 512):
            qw = min(512, w - q)
            nc.tensor.matmul(
                g[:, q : q + qw],
                lhsT=vn_bf[:, b * 128 : (b + 1) * 128],
                rhs=vn_bf[:, c0 + q : c0 + q + qw],
                start=True,
                stop=True,
            )
        if ci + 1 < nch:
            # prefetch the next chunk's vn AFTER this gram (so the gram's
            # counter-sem threshold excludes it) but BEFORE this exp (so
            # the ln/rinv execute under it instead of serializing after)
            nb_, nc0, nw = chunks[ci + 1][0:3]
            ensure_vn(nc0 + nw)
        if not is_poly[ci]:
            e = e_pool.tile([128, CW], BF16, tag="e", name=f"e{ci}")
            nc.scalar.activation(e[:, :w], g[:, :w], AF.Exp)
            e_tiles[ci] = e
            emit_transpose(ci)

    def emit_poly(ci):
        # deferred ~2 iterations past the gram so the DVE work doesn't
        # inflate the counter-semaphore thresholds of the next grams;
        # the final square runs on the idle Pool engine (or split DVE/Pool)
        b, c0, w, k0, nk, _pol = chunks[ci]
        g = g_tiles[ci]
        y = pp.tile([128, CW], BF16, tag="py", name=f"py{ci}")
        nc.vector.tensor_scalar(
            out=y[:, :w], in0=g[:, :w], scalar1=0.5, scalar2=None,
            op0=ALU.mult,
        )
        u = pp.tile([128, CW], BF16, tag="pu", name=f"pu{ci}")
        nc.vector.scalar_tensor_tensor(
            out=u[:, :w], in0=y[:, :w], scalar=PS2, in1=y[:, :w],
            op0=ALU.add, op1=ALU.mult,
        )
        p = pp.tile([128, CW], BF16, tag="ppt", name=f"ppt{ci}")
        nc.vector.tensor_scalar(
            out=p[:, :w], in0=u[:, :w], scalar1=PC2, scalar2=PA,
            op0=ALU.mult, op1=ALU.add,
        )
        p_tiles[ci] = p

    def emit_poly_sq(ci):
        # square + transpose one slot later still: keeps the Pool-counter
        # thresholds of later-emitted exps a pass older in the thin-window
        # tail region
        b, c0, w, k0, nk, _pol = chunks[ci]
        p = p_tiles[ci]
        e = e_pool.tile([128, CW], BF16, tag="e", name=f"e{ci}")
        wd = int(w * _square_split(b)) // 128 * 128
        if wd > 0:
            nc.vector.tensor_mul(e[:, :wd], p[:, :wd], p[:, :wd])
        if wd < w:
            nc.gpsimd.tensor_mul(e[:, wd:w], p[:, wd:w], p[:, wd:w])
        e_tiles[ci] = e
        emit_transpose(ci)

    def emit_easy(ci):
        b, c0, w, k0, nk, _pol = chunks[ci]
        ensure_oh(b)
        for t in range(nk):
            k = k0 + t
            st, sp = cst_flags(k)
            nc.tensor.matmul(
                cst_slice(k),
                lhsT=e_tiles[ci][:, t * 128 : (t + 1) * 128],
                rhs=oh_slice(b),
                start=st,
                stop=sp,
            )

    def emit_hard(ci):
        b, c0, w, k0, nk, _pol = chunks[ci]
        ntb = nk - 1 if k0 == b else nk
        kfirst = b + 1 if k0 == b else k0
        ensure_oh(kfirst + ntb - 1)
        for t in range(ntb):
            k = kfirst + t
            st, sp = cst_flags(b)
            nc.tensor.matmul(
                cst_slice(b),
                lhsT=et_tiles[ci][:, t * 128 : (t + 1) * 128],
                rhs=oh_slice(k),
                start=st,
                stop=sp,
            )

    # ---------------- phased epilogue: sel/tot per 16-block half -----------
    masked = sb.tile([128, NB * NCLS], F32, tag="masked")
    sel = sb.tile([128, NB], F32, tag="sel")
    tot = sb.tile([128, NB], F32, tag="tot")

    a_t = sb.tile([128, NB], F32, tag="a_t")
    b_t = sb.tile([128, NB], F32, tag="b_t")
    num = sb.tile([128, NB], F32, tag="num")
    den = sb.tile([128, NB], F32, tag="den")
    l_den = sb.tile([128, NB], F32, tag="l_den")
    l_num = sb.tile([128, NB], F32, tag="l_num")
    lt = sb.tile([128, NB], F32, tag="lt")

    def _seltot_chores(h):
        nblk = CSPLIT if h == 0 else NB - CSPLIT
        lo, hi = (0, CSPLIT * NCLS) if h == 0 else (CSPLIT * NCLS, NB * NCLS)
        bs = slice(0, CSPLIT) if h == 0 else slice(CSPLIT, NB)
        cs = cst_t[h][:, 0 : nblk * NCLS]
        yield lambda: nc.vector.tensor_mul(masked[:, lo:hi], cs, oh_f[:, lo:hi])
        yield lambda: nc.vector.tensor_reduce(
            out=sel[:, bs],
            in_=masked[:, lo:hi].rearrange("p (b c) -> p b c", c=NCLS),
            axis=AX.X,
            op=ALU.add,
        )
        yield lambda: nc.vector.tensor_reduce(
            out=tot[:, bs],
            in_=cs.rearrange("p (b c) -> p b c", c=NCLS),
            axis=AX.X,
            op=ALU.add,
        )
        nblk2 = CSPLIT if h == 0 else NB - CSPLIT
        nd = sb.tile([128, 2 * NB], F32, tag=f"nd{h}", name=f"nd{h}")
        lnd = sb.tile([128, 2 * NB], F32, tag=f"lnd{h}", name=f"lnd{h}")
        # num = (sel - e) * nbar in one fused stt; num/den land in adjacent
        # column ranges of one tile so a single Ln covers both
        yield lambda: nc.vector.scalar_tensor_tensor(
            out=nd[:, 0:nblk2], in0=sel[:, bs], scalar=-E_CONST,
            in1=nbar[:, bs], op0=ALU.add, op1=ALU.mult,
        )
        yield lambda: nc.vector.tensor_sub(b_t[:, bs], tot[:, bs], sel[:, bs])
        def fin1():
            nc.vector.tensor_mul(nd[:, nblk2 : 2 * nblk2], b_t[:, bs], n_row[:, bs])
            nc.vector.tensor_add(
                nd[:, nblk2 : 2 * nblk2], nd[:, nblk2 : 2 * nblk2], nd[:, 0:nblk2]
            )
        yield fin1
        def fin2():
            nc.scalar.activation(lnd[:, 0 : 2 * nblk2], nd[:, 0 : 2 * nblk2], AF.Ln)
            nc.vector.tensor_sub(
                lt[:, bs], lnd[:, nblk2 : 2 * nblk2], lnd[:, 0:nblk2]
            )
            nc.sync.dma_start(out=outp[:, bs], in_=lt[:, bs])
        yield fin2

    def emit_seltot(h):
        for chore in _seltot_chores(h):
            chore()

    ensure_oh(0)  # block 0's one-hot before any vn work lands on DVE

    # Deferred cs-matmul schedule: poly chunks produce e ~3us later than
    # ACT chunks, so their easy/hard emission is pushed further out to keep
    # the in-order PE queue from blocking the next gram (which stalls ACT).
    # Deferral is measured in ACT chunks, not raw chunk index: easy(cj) must
    # sit behind >= 2 further exps (and poly output behind the DVE latency)
    # when PE reaches it, else its wait blocks the next gram and stalls ACT.
    NSLOT = nch + 8
    easy_due = [[] for _ in range(NSLOT)]
    hard_due = [[] for _ in range(NSLOT)]
    act_idx = [ci for ci in range(nch) if not is_poly[ci]]
    hard_slot_of = {}
    for cj in range(nch):
        later = [ci for ci in act_idx if ci > cj]
        ne, nh = (2, 5) if is_poly[cj] else (1, 4)
        easy_slot = later[ne] if len(later) > ne else nch + 1
        hard_slot = later[nh] if len(later) > nh else nch + 2
        easy_due[easy_slot].append(cj)
        hard_due[hard_slot].append(cj)
        hard_slot_of[cj] = hard_slot

    # bank 0 (blocks 0-15) closes once the hard matmuls of the last chunk
    # of pass 15 are emitted
    last_chunk_of_pass = {}
    for ci, (b, _, _, _, _, _pol) in enumerate(chunks):
        last_chunk_of_pass[b] = ci
    half0_at = hard_slot_of[last_chunk_of_pass[CSPLIT - 1]]

    poly_due = [[] for _ in range(NSLOT)]
    polysq_due = [[] for _ in range(NSLOT)]
    for cj in range(nch):
        if is_poly[cj]:
            poly_due[cj + 2].append(cj)
            polysq_due[cj + 3].append(cj)

    chores = list(_counts_chores())
    chore_pos = 0

    for ci in range(NSLOT):
        if ci < nch:
            emit_gram_exp(ci)
        for cj in poly_due[ci]:
            emit_poly(cj)
        for cj in polysq_due[ci]:
            emit_poly_sq(cj)
        if ci >= 6:
            for _ in range(2):
                if chore_pos < len(chores):
                    chores[chore_pos]()
                    chore_pos += 1
        for cj in easy_due[ci]:
            emit_easy(cj)
        for cj in hard_due[ci]:
            if et_tiles[cj] is not None:
                emit_hard(cj)
        if ci == half0_at:
            chores.extend(_seltot_chores(0))
    while chore_pos < len(chores):
        chores[chore_pos]()
        chore_pos += 1
    emit_seltot(1)


def build_nc():
    nc = bacc.Bacc()
    feat = nc.declare_dram_parameter("features", [C, N], F32, isOutput=False)
    lab = nc.declare_dram_parameter("labels", [128, NB], F32, isOutput=False)
    outp = nc.declare_dram_parameter("out", [128, NB], F32, isOutput=True)
    with tile.TileContext(nc) as tc:
        with ExitStack() as ctx:
            _body(ctx, tc, feat[:, :], lab[:, :], outp)
    nc.finalize()
    return nc


_NC_CACHE = None


def _get_nc():
    global _NC_CACHE
    if _NC_CACHE is None:
        _NC_CACHE = build_nc()
    return _NC_CACHE


def make_in_maps(features: np.ndarray, labels_all: np.ndarray):
    in_maps = []
    for i in range(B):
        f = np.ascontiguousarray(features[i], dtype=np.float32)
        # labels_sb[p, b] = labels[128*b + p]
        l = np.ascontiguousarray(
            labels_all[i].astype(np.float32).reshape(NB, 128).T
        )
        in_maps.append({"features": f, "labels": l})
    return in_maps


def kernel(features: np.ndarray, labels_all: np.ndarray) -> np.ndarray:
    nc = _get_nc()
    in_maps = make_in_maps(features, labels_all)
    r = run_bass_kernel_spmd(nc, in_maps, core_ids=list(range(B)))
    sums = np.array(
        [np.sum(r.results[i]["out"], dtype=np.float64) for i in range(B)]
    )
    return np.float32(np.mean(sums) / N)
